# revision 31
# baseline (speedup 1.0000x reference)
"""CapsuleLayer dynamic-routing kernel for Trainium2 (Bass/Tile), SPMD over 8 cores.

Math (per batch sample, from the reference):
    u_hat[j, (i,k)] = sum_k' x[j, k'] * W[k', (i,k)]        j=1024, k'=256, (i,k)=16x32=512
    b_0 = 0
    for t in 0..3:
        c = softmax_i(b)                                    [16, 1024]
        s[i, k] = sum_j c[i, j] * u_hat[j, (i,k)]
        v = s / sqrt(sum_k s^2 + eps)                       [16, 32]
        if t < 3: b[i, j] = sum_k v[i, k] * u_hat[j, (i,k)]
    return v

Sharding: data-parallel over batch (128 -> 16 per core), W replicated.

v3 layout strategy (all matmul inputs fp16, PSUM fp32):
  - x is pre-transposed AND pre-summed over j on the HOST: xT fp16 [256, 1024]
    uploads with plain DMAs (no device xbar transposes, no startup serialization).
  - u_hat [j-part, (ik)] via matmul lhsT=xT-chunks rhs=W; PSUM evacuated
    fp32->fp16 alternating Vector/Scalar.
  - u_hatT [(ik)-part, j] via ONE SBUF->SBUF DMA-crossbar transpose per sample
    on the sync HWDGE queue (scalar-queue DMAs block ScalarE compute); group 0
    uses PE transposes instead since the sync queue can't fill fast enough at
    startup.
  - routing iteration 0 exploits b=0 => c uniform: s_0 = (xsum @ W)/16, done as a
    single full-bank matmul against a host-built replicated-xsum stationary (XS);
    no c-tile init is needed anywhere.
  - routing processes 4 samples per group, packed 32-per-sample in PSUM
    partitions with col-group tile_position for concurrent PE strips; one
    accumulation group per bank (start=True only on the very first matmul).
  - every 32-row strip is its OWN PSUM accumulation group (start=True on its
    first matmul clears has_written for just that strip region), so no PSUM
    memsets / zero-inits are needed; junk rows (16..31 of each strip) stay
    finite and self-damping (exp of junk is exp(rinv*0)=1, the Z-sum only
    reduces real columns, and the mask zeroes junk before anything nonlinear),
    so no NaN/inf can ever contaminate real lanes.
  - rsqrt via DVE magic-constant + ONE Newton step (0.17% max err, well within
    tolerance); keeping Ln off ScalarE avoids activation-table thrash (only one
    table load for Copy/Square/Exp).
  - softmax runs in the bT layout [j-part, (sample, i)]; 1/Z applied as one flat
    [128,1024] multiply against a broadcast-expanded reciprocal.
  - final v written packed [128, 32] per group with one contiguous DMA to a
    DRAM scratch; host unpacks strips.
"""

import functools

import numpy as np

import concourse.bass as bass
import concourse.mybir as mybir
import concourse.tile as tile
from concourse import bacc
from concourse.bass_utils import run_bass_kernel_spmd

F32 = mybir.dt.float32
F16 = mybir.dt.float16
I32 = mybir.dt.int32
AF = mybir.ActivationFunctionType
ALU = mybir.AluOpType
AX = mybir.AxisListType
ts = bass.ts

NCORES = 8
BFULL = 128
BSH = BFULL // NCORES  # 16 samples per core
NJ, NK, ND = 1024, 256, 512  # j, k', (i,k)
NI, DK = 16, 32
JT, KT, IKT = NJ // 128, NK // 128, ND // 128  # 8, 2, 4
GS = 4  # samples per routing group (packed in PSUM partitions at 32-stride)
NG = BSH // GS  # 4
ROUTINGS = 4
EPS = 1e-7
P = 128


def _build_body(nc, tc, xT_ap, w_ap, xs_ap, ident_ap, sel_ap, mask_ap, out_ap, ctx):
    consts = ctx.enter_context(tc.tile_pool(name="consts", bufs=1))
    xTp = ctx.enter_context(tc.tile_pool(name="xT", bufs=6))
    uhp = ctx.enter_context(tc.tile_pool(name="uh", bufs=2 * GS))
    uhTp = ctx.enter_context(tc.tile_pool(name="uhT", bufs=2 * GS))
    rt = ctx.enter_context(tc.tile_pool(name="rt", bufs=3))
    sm = ctx.enter_context(tc.tile_pool(name="sm", bufs=3))
    psum = ctx.enter_context(tc.tile_pool(name="psum", bufs=2, space="PSUM"))

    # ---- input/const DMAs ----
    # gpsimd queue: all x uploads + most consts, in consumption order.
    # sync queue: W (gates the first matmul), then the even-sample uhT
    # transposes; scalar queue: odd-sample uhT transposes.  Keeping the
    # transposes off the x-upload queue and in consumption order avoids
    # head-of-line blocking on the HWDGE queues.
    xT_tiles = []
    for s in range(BSH):
        xTt = xTp.tile([P, KT, NJ], F16, name="xT")
        xT_tiles.append(xTt)

    wf = consts.tile([P, KT, ND], F16)
    ident = consts.tile([P, P], F16)
    sel = consts.tile([P, DK], F16)
    mask = consts.tile([P, ND], F16)
    XSt = consts.tile([P, NG, KT, P], F16)

    nc.sync.dma_start(wf[:], w_ap.rearrange("(t p) d -> p t d", p=P))
    nc.gpsimd.dma_start(
        xT_tiles[0][:, :, 0:P], xT_ap[0].rearrange("(t p) j -> p t j", p=P)[:, :, 0:P]
    )
    nc.gpsimd.dma_start(
        xT_tiles[0][:, :, P:NJ], xT_ap[0].rearrange("(t p) j -> p t j", p=P)[:, :, P:NJ]
    )
    for s in range(1, 4):
        nc.gpsimd.dma_start(
            xT_tiles[s][:], xT_ap[s].rearrange("(t p) j -> p t j", p=P)
        )
    nc.gpsimd.dma_start(ident[:], ident_ap)
    nc.gpsimd.dma_start(mask[:], mask_ap)
    nc.gpsimd.dma_start(XSt[:], xs_ap)
    nc.gpsimd.dma_start(sel[:], sel_ap)
    for s in range(4, BSH):
        nc.gpsimd.dma_start(
            xT_tiles[s][:], xT_ap[s].rearrange("(t p) j -> p t j", p=P)
        )

    uh_tiles = [None] * BSH
    uhT_tiles = [None] * BSH

    def stage1(s):
        """xT[s] -> u_hat [j,(ik)] fp16 -> u_hatT via DMA xbar transpose."""
        xTt = xT_tiles[s]
        uh = uhp.tile([P, JT, ND], F16, name="uh")
        for jt in range(JT):
            pu = psum.tile([P, ND], F32, name="pu", tag="pu", bufs=3)
            for kt in range(KT):
                nc.tensor.matmul(
                    pu[:],
                    lhsT=xTt[:, kt, ts(jt, P)],
                    rhs=wf[:, kt, :],
                    start=(kt == 0),
                    stop=(kt == KT - 1),
                )
            if jt % 2 == 0:
                nc.vector.tensor_copy(uh[:, jt, :], pu[:])
            else:
                nc.scalar.copy(uh[:, jt, :], pu[:])

        # uhT[p, jt, dt, jl] = u_hat[128*jt + jl, 128*dt + p]
        uhT = uhTp.tile([P, JT, IKT, P], F16, name="uhT")
        if s < GS:
            # group 0: PE transposes (sync DMA queue can't fill fast enough
            # at startup); evac alternates Vector/Scalar
            for dt in range(IKT):
                pt2 = psum.tile([P, NJ], F16, name="pt2", tag="pf16", bufs=2)
                for jt in range(JT):
                    nc.tensor.transpose(
                        pt2[:, ts(jt, P)], uh[:, jt, ts(dt, P)], ident[:]
                    )
                if dt % 2 == 0:
                    nc.vector.tensor_copy(uhT[:, :, dt, :], pt2.rearrange("p (t c) -> p t c", c=P))
                else:
                    nc.scalar.copy(uhT[:, :, dt, :], pt2.rearrange("p (t c) -> p t c", c=P))
        else:
            # two jc-halves back to back: no head-of-line blocking (each half
            # only needs this sample's uh), and the jc=0 b-update can start
            # as soon as every sample's A-half is done.
            nc.sync.dma_start_transpose(
                uhT[:, 0:4, :, :].rearrange("p a b c -> p (a b) c"),
                uh[:, 0:4, :].rearrange("p t d -> p (t d)"),
            )
            nc.sync.dma_start_transpose(
                uhT[:, 4:8, :, :].rearrange("p a b c -> p (a b) c"),
                uh[:, 4:8, :].rearrange("p t d -> p (t d)"),
            )
        uh_tiles[s] = uh
        uhT_tiles[s] = uhT

    ct_state = {}

    def routing_iter(g, t):
        samples = [g * GS + i for i in range(GS)]
        if t == 0:
            # two cT tiles, alternating across iterations
            ct_state[g] = [
                sm.tile([P, JT, GS, 32], F16, name="ct", tag="ct", bufs=4)
                for _ in range(2)
            ]
        ct_tiles = ct_state[g]
        if True:
            # s-einsum: 4 samples concurrent in one PSUM bank via col groups.
            ps_s = psum.tile([P, ND], F32, name="ps_s", tag="prt", bufs=3)
            if t == 0:
                # b=0 => c uniform: s_0 = (xsum @ W)/16 replicated over strips
                for kt in range(KT):
                    nc.tensor.matmul(
                        ps_s[:],
                        lhsT=XSt[:, g, kt, :],
                        rhs=wf[:, kt, :],
                        start=(kt == 0),
                        stop=(kt == KT - 1),
                    )
            else:
                ct = ct_tiles[t % 2]
                for jt in range(JT):
                    for a in range(GS):
                        nc.tensor.matmul(
                            ps_s[ts(a, 32), :],
                            lhsT=ct[:, jt, a, :],
                            rhs=uh_tiles[samples[a]][:, jt, :],
                            start=(jt == 0),
                            stop=(jt == JT - 1),
                            tile_position=(0, 32 * a),
                            skip_group_check=True,
                        )

            # mask to block diagonal; norms; rinv = (n2+eps)^-0.5 on ScalarE as
            # Exp(-0.5 * Ln(n2+eps)); junk rows produce garbage that stays put.
            masked = rt.tile([P, ND], F16, name="masked")
            for mh in range(2):
                nc.vector.tensor_tensor(
                    masked[:, ts(mh, 256)], ps_s[:, ts(mh, 256)],
                    mask[:, ts(mh, 256)], op=ALU.mult,
                )
            sq = rt.tile([P, ND], F16, name="sq")
            n2 = rt.tile([P, 1], F32, name="n2")
            nc.scalar.activation(sq[:], masked[:], AF.Square, accum_out=n2[:])
            # rinv = (n2+eps)^-0.5: magic-constant guess + 1 Newton step on DVE
            xe = rt.tile([P, 1], F32, name="xe")
            nc.vector.tensor_scalar(xe[:], n2[:], EPS, None, op0=ALU.add)
            xh = rt.tile([P, 1], F32, name="xh")
            nc.vector.tensor_scalar(xh[:], xe[:], 0.5, None, op0=ALU.mult)
            yt = rt.tile([P, 1], F32, name="yt")
            nc.vector.tensor_scalar(
                yt.bitcast(I32)[:], xe.bitcast(I32)[:], 1, None,
                op0=ALU.logical_shift_right,
            )
            nc.vector.tensor_scalar(
                yt.bitcast(I32)[:], yt.bitcast(I32)[:], 0x5F3759E0, None,
                op0=ALU.subtract,
            )
            nc.vector.tensor_scalar(
                yt.bitcast(I32)[:], yt.bitcast(I32)[:], -1, None,
                op0=ALU.bitwise_xor,
            )
            y2 = rt.tile([P, 1], F32, name="y2")
            nc.vector.tensor_tensor(y2[:], yt[:], yt[:], op=ALU.mult)
            nc.vector.tensor_tensor(y2[:], y2[:], xh[:], op=ALU.mult)
            nc.vector.tensor_scalar(y2[:], y2[:], -1.0, 1.5, op0=ALU.mult, op1=ALU.add)
            nc.vector.tensor_tensor(yt[:], yt[:], y2[:], op=ALU.mult)
            rinv = yt

            # block-diagonal V (unnormalized): PE transpose of masked
            pv = psum.tile([P, IKT * P], F16, name="pv", tag="pf16", bufs=2)
            for c in range(IKT):
                nc.tensor.transpose(pv[:, ts(c, P)], masked[:, ts(c, P)], ident[:])
            vblk = rt.tile([P, IKT, P], F16, name="vblk")
            for c in range(IKT):
                nc.scalar.copy(vblk[:, c, :], pv[:, ts(c, P)])

            if t == ROUTINGS - 1:
                # final squash output: diag-extract via matmul with Sel, scale,
                # write packed [128, 32]; host unpacks the 4 strips.
                ps_v = psum.tile([P, DK], F32, name="ps_v", tag="prt", bufs=3)
                for kt in range(IKT):
                    nc.tensor.matmul(
                        ps_v[:],
                        lhsT=vblk[:, kt, :],
                        rhs=sel[:],
                        start=(kt == 0),
                        stop=(kt == IKT - 1),
                    )
                vout = rt.tile([P, DK], F32, name="vout")
                nc.scalar.activation(vout[:], ps_v[:], AF.Copy, scale=rinv[:])
                nc.gpsimd.dma_start(out_ap[g], vout[:])
                return

            # b-update: b[i,j] = sum_k v u_hatT; exp(rinv*b) fused into the PSUM
            # evacuation. Junk rows exp to garbage; it never leaves them.
            bsc = rt.tile([P, 2, ND], F16, name="bsc")
            for jc in range(2):
                ps_b = psum.tile([P, ND], F32, name="ps_b", tag="prt", bufs=3)
                for kt in range(IKT):
                    for a in range(GS):
                        nc.tensor.matmul(
                            ps_b[ts(a, 32), :],
                            lhsT=vblk[:, kt, ts(a, 32)],
                            rhs=uhT_tiles[samples[a]][:, 4 * jc : 4 * jc + 4, kt, :],
                            start=(kt == 0),
                            stop=(kt == IKT - 1),
                            tile_position=(0, 32 * a),
                            skip_group_check=True,
                        )
                for eh in range(2):
                    nc.scalar.activation(
                        bsc[:, jc, ts(eh, 256)], ps_b[:, ts(eh, 256)],
                        AF.Exp, scale=rinv[:],
                    )

            # transpose to bT [j-part, (sample, i)] and softmax over i;
            # processed in jc-halves so half 0's normalize chain overlaps
            # half 1's exp + transposes (cuts the serial tail per iteration)
            pbt = psum.tile([P, JT, P], F16, name="pbt", tag="pf16", bufs=2)
            expT = pbt.rearrange("p t (s c) -> p t s c", c=32)[:, :, :, 0:NI]
            zsum = sm.tile([P, JT, GS], F32, name="zsum")
            rz = sm.tile([P, JT, GS], F32, name="rz")
            rzx = sm.tile([P, JT, GS, 32], F16, name="rzx")
            ct_next = ct_tiles[(t + 1) % 2]
            ctv = ct_next.rearrange("p t s c -> p t (s c)")
            for h in range(2):
                hs = slice(4 * h, 4 * h + 4)
                for jt in range(4 * h, 4 * h + 4):
                    nc.tensor.transpose(
                        pbt[:, jt, :], bsc[:, jt // 4, ts(jt % 4, P)], ident[:]
                    )
                nc.vector.tensor_reduce(
                    zsum[:, hs, :], expT[:, hs, :, :], axis=AX.X, op=ALU.add
                )
                nc.vector.reciprocal(rz[:, hs, :], zsum[:, hs, :])
                nc.vector.tensor_copy(
                    rzx[:, hs, :, :],
                    rz[:, hs, :].unsqueeze(3).broadcast_to([P, 4, GS, 32]),
                )
                nc.vector.tensor_tensor(
                    ctv[:, hs, :],
                    pbt[:, hs, :],
                    rzx.rearrange("p t s c -> p t (s c)")[:, hs, :],
                    op=ALU.mult,
                )

    # software-pipelined emission: engine queues are in-order, so emission
    # order is the schedule. stage1 of the next group is spread between
    # routing iterations; the last two groups' iterations are interleaved.
    for s in range(GS):
        stage1(s)
    sched = [
        ("r", 0, 0), ("s", 4), ("s", 5), ("r", 0, 1), ("s", 6), ("s", 7),
        ("r", 0, 2), ("r", 0, 3),
        ("s", 8), ("s", 9), ("r", 1, 0), ("s", 10), ("s", 11), ("r", 1, 1),
        ("r", 1, 2), ("r", 1, 3),
        ("s", 12), ("s", 13), ("r", 2, 0), ("s", 14), ("s", 15), ("r", 2, 1),
        ("r", 2, 2), ("r", 3, 0), ("r", 2, 3), ("r", 3, 1),
        ("r", 3, 2), ("r", 3, 3),
    ]
    for item in sched:
        if item[0] == "s":
            stage1(item[1])
        else:
            routing_iter(item[1], item[2])


def _np_consts():
    ident = np.eye(P, dtype=np.float16)
    sel = np.tile(np.eye(DK, dtype=np.float16), (IKT, 1))
    mask = np.zeros((P, ND), dtype=np.float16)
    for a in range(GS):
        for i in range(NI):
            mask[32 * a + i, DK * i : DK * (i + 1)] = 1.0
    return ident, sel, mask


@functools.cache
def _build_nc():
    from contextlib import ExitStack

    nc = bacc.Bacc(
        "TRN2",
        target_bir_lowering=False,
        debug=False,
        num_devices=NCORES,
    )
    xT_t = nc.dram_tensor("xT", [BSH, NK, NJ], F16, kind="ExternalInput")
    w_t = nc.dram_tensor("w", [NK, ND], F16, kind="ExternalInput")
    xs_t = nc.dram_tensor("xs", [P, NG, KT, P], F16, kind="ExternalInput")
    ident_t = nc.dram_tensor("ident", [P, P], F16, kind="ExternalInput")
    sel_t = nc.dram_tensor("sel", [P, DK], F16, kind="ExternalInput")
    mask_t = nc.dram_tensor("mask", [P, ND], F16, kind="ExternalInput")
    out_t = nc.dram_tensor("out", [NG, P, DK], F32, kind="ExternalOutput")

    with tile.TileContext(nc) as tc:
        with ExitStack() as ctx:
            _build_body(
                nc, tc,
                xT_t.ap(), w_t.ap(), xs_t.ap(), ident_t.ap(), sel_t.ap(),
                mask_t.ap(), out_t.ap(),
                ctx,
            )
    nc.compile()
    return nc


def _in_maps(x, W):
    x = np.asarray(x, dtype=np.float32)
    w2d = np.asarray(W, dtype=np.float32).reshape(NK, ND).astype(np.float16)
    ident, sel, mask = _np_consts()
    maps = []
    for c in range(NCORES):
        shard = x[c * BSH : (c + 1) * BSH]
        xT = np.ascontiguousarray(shard.transpose(0, 2, 1)).astype(np.float16)
        xsum = shard.sum(axis=1)  # [BSH, NK] fp32
        XS = np.zeros((P, NG, KT, P), np.float32)
        for g in range(NG):
            for a in range(GS):
                col = xsum[g * GS + a].reshape(KT, P).transpose(1, 0) / NI
                XS[:, g, :, 32 * a : 32 * a + NI] = col[:, :, None]
        maps.append(
            {
                "xT": xT,
                "w": np.ascontiguousarray(w2d),
                "xs": XS.astype(np.float16),
                "ident": ident,
                "sel": sel,
                "mask": mask,
            }
        )
    return maps


def run(x, W, trace=False):
    nc = _build_nc()
    res = run_bass_kernel_spmd(nc, _in_maps(x, W), list(range(NCORES)), trace=trace)
    outs = []
    for r in res.results:
        scr = r["out"]  # [NG, 128, 32]
        v = scr.reshape(NG, GS, 32, DK)[:, :, :NI, :].reshape(BSH, NI, DK)
        outs.append(v)
    out = np.concatenate(outs, axis=0)
    return out.astype(np.float32), res


def kernel(x, W):
    out, _ = run(x, W, trace=False)
    return out


# revision 32
# speedup vs baseline: 1.0362x; 1.0362x over previous
"""CapsuleLayer dynamic-routing kernel for Trainium2 (Bass/Tile), SPMD over 8 cores.

Math (per batch sample, from the reference):
    u_hat[j, (i,k)] = sum_k' x[j, k'] * W[k', (i,k)]        j=1024, k'=256, (i,k)=16x32=512
    b_0 = 0
    for t in 0..3:
        c = softmax_i(b)                                    [16, 1024]
        s[i, k] = sum_j c[i, j] * u_hat[j, (i,k)]
        v = s / sqrt(sum_k s^2 + eps)                       [16, 32]
        if t < 3: b[i, j] = sum_k v[i, k] * u_hat[j, (i,k)]
    return v

Sharding: data-parallel over batch (128 -> 16 per core), W replicated.

v3 layout strategy (all matmul inputs fp16, PSUM fp32):
  - x is pre-transposed AND pre-summed over j on the HOST: xT fp16 [256, 1024]
    uploads with plain DMAs (no device xbar transposes, no startup serialization).
  - u_hat [j-part, (ik)] via matmul lhsT=xT-chunks rhs=W; PSUM evacuated
    fp32->fp16 alternating Vector/Scalar.
  - u_hatT [(ik)-part, j] via ONE SBUF->SBUF DMA-crossbar transpose per sample
    on the sync HWDGE queue (scalar-queue DMAs block ScalarE compute); group 0
    uses PE transposes instead since the sync queue can't fill fast enough at
    startup.
  - routing iteration 0 exploits b=0 => c uniform: s_0 = (xsum @ W)/16, done as a
    single full-bank matmul against a host-built replicated-xsum stationary (XS);
    no c-tile init is needed anywhere.
  - routing processes 4 samples per group, packed 32-per-sample in PSUM
    partitions with col-group tile_position for concurrent PE strips; one
    accumulation group per bank (start=True only on the very first matmul).
  - every 32-row strip is its OWN PSUM accumulation group (start=True on its
    first matmul clears has_written for just that strip region), so no PSUM
    memsets / zero-inits are needed; junk rows (16..31 of each strip) stay
    finite and self-damping (exp of junk is exp(rinv*0)=1, the Z-sum only
    reduces real columns, and the mask zeroes junk before anything nonlinear),
    so no NaN/inf can ever contaminate real lanes.
  - rsqrt via DVE magic-constant + ONE Newton step (0.17% max err, well within
    tolerance); keeping Ln off ScalarE avoids activation-table thrash (only one
    table load for Copy/Square/Exp).
  - softmax runs in the bT layout [j-part, (sample, i)]; 1/Z applied as one flat
    [128,1024] multiply against a broadcast-expanded reciprocal.
  - final v written packed [128, 32] per group with one contiguous DMA to a
    DRAM scratch; host unpacks strips.
"""

import functools

import numpy as np

import concourse.bass as bass
import concourse.mybir as mybir
import concourse.tile as tile
from concourse import bacc
from concourse.bass_utils import run_bass_kernel_spmd

F32 = mybir.dt.float32
F16 = mybir.dt.float16
I32 = mybir.dt.int32
AF = mybir.ActivationFunctionType
ALU = mybir.AluOpType
AX = mybir.AxisListType
ts = bass.ts

NCORES = 8
BFULL = 128
BSH = BFULL // NCORES  # 16 samples per core
NJ, NK, ND = 1024, 256, 512  # j, k', (i,k)
NI, DK = 16, 32
JT, KT, IKT = NJ // 128, NK // 128, ND // 128  # 8, 2, 4
GS = 4  # samples per routing group (packed in PSUM partitions at 32-stride)
NG = BSH // GS  # 4
ROUTINGS = 4
EPS = 1e-7
P = 128


def _build_body(nc, tc, xT_ap, w_ap, xs_ap, ident_ap, sel_ap, mask_ap, out_ap, ctx):
    consts = ctx.enter_context(tc.tile_pool(name="consts", bufs=1))
    xTp = ctx.enter_context(tc.tile_pool(name="xT", bufs=6))
    uhp = ctx.enter_context(tc.tile_pool(name="uh", bufs=2 * GS))
    uhTp = ctx.enter_context(tc.tile_pool(name="uhT", bufs=2 * GS))
    rt = ctx.enter_context(tc.tile_pool(name="rt", bufs=3))
    sm = ctx.enter_context(tc.tile_pool(name="sm", bufs=3))
    psum = ctx.enter_context(tc.tile_pool(name="psum", bufs=2, space="PSUM"))

    # ---- input/const DMAs ----
    # gpsimd queue: all x uploads + most consts, in consumption order.
    # sync queue: W (gates the first matmul), then the even-sample uhT
    # transposes; scalar queue: odd-sample uhT transposes.  Keeping the
    # transposes off the x-upload queue and in consumption order avoids
    # head-of-line blocking on the HWDGE queues.
    xT_tiles = []
    for s in range(BSH):
        xTt = xTp.tile([P, KT, NJ], F16, name="xT")
        xT_tiles.append(xTt)

    wf = consts.tile([P, KT, ND], F16)
    ident = consts.tile([P, P], F16)
    sel = consts.tile([P, DK], F16)
    mask = consts.tile([P, ND], F16)
    XSt = consts.tile([P, NG, KT, P], F16)

    nc.sync.dma_start(wf[:], w_ap.rearrange("(t p) d -> p t d", p=P))
    for s in range(4):
        nc.gpsimd.dma_start(
            xT_tiles[s][:], xT_ap[s].rearrange("(t p) j -> p t j", p=P)
        )
    nc.gpsimd.dma_start(ident[:], ident_ap)
    nc.gpsimd.dma_start(mask[:], mask_ap)
    nc.gpsimd.dma_start(XSt[:], xs_ap)
    nc.gpsimd.dma_start(sel[:], sel_ap)
    for s in range(4, BSH):
        nc.gpsimd.dma_start(
            xT_tiles[s][:], xT_ap[s].rearrange("(t p) j -> p t j", p=P)
        )

    uh_tiles = [None] * BSH
    uhT_tiles = [None] * BSH

    def stage1(s):
        """xT[s] -> u_hat [j,(ik)] fp16 -> u_hatT via DMA xbar transpose."""
        xTt = xT_tiles[s]
        uh = uhp.tile([P, JT, ND], F16, name="uh")
        for jt in range(JT):
            pu = psum.tile([P, ND], F32, name="pu", tag="pu", bufs=3)
            for kt in range(KT):
                nc.tensor.matmul(
                    pu[:],
                    lhsT=xTt[:, kt, ts(jt, P)],
                    rhs=wf[:, kt, :],
                    start=(kt == 0),
                    stop=(kt == KT - 1),
                )
            if jt % 2 == 0:
                nc.vector.tensor_copy(uh[:, jt, :], pu[:])
            else:
                nc.scalar.copy(uh[:, jt, :], pu[:])

        # uhT[p, jt, dt, jl] = u_hat[128*jt + jl, 128*dt + p]
        uhT = uhTp.tile([P, JT, IKT, P], F16, name="uhT")
        if s < GS:
            # group 0: PE transposes (sync DMA queue can't fill fast enough
            # at startup); evac alternates Vector/Scalar
            for dt in range(IKT):
                pt2 = psum.tile([P, NJ], F16, name="pt2", tag="pf16", bufs=2)
                for jt in range(JT):
                    nc.tensor.transpose(
                        pt2[:, ts(jt, P)], uh[:, jt, ts(dt, P)], ident[:]
                    )
                if dt % 2 == 0:
                    nc.vector.tensor_copy(uhT[:, :, dt, :], pt2.rearrange("p (t c) -> p t c", c=P))
                else:
                    nc.scalar.copy(uhT[:, :, dt, :], pt2.rearrange("p (t c) -> p t c", c=P))
        else:
            # two jc-halves back to back: no head-of-line blocking (each half
            # only needs this sample's uh), and the jc=0 b-update can start
            # as soon as every sample's A-half is done.
            nc.sync.dma_start_transpose(
                uhT[:, 0:4, :, :].rearrange("p a b c -> p (a b) c"),
                uh[:, 0:4, :].rearrange("p t d -> p (t d)"),
            )
            nc.sync.dma_start_transpose(
                uhT[:, 4:8, :, :].rearrange("p a b c -> p (a b) c"),
                uh[:, 4:8, :].rearrange("p t d -> p (t d)"),
            )
        uh_tiles[s] = uh
        uhT_tiles[s] = uhT

    ct_state = {}

    def routing_iter(g, t):
        samples = [g * GS + i for i in range(GS)]
        if t == 0:
            # two cT tiles, alternating across iterations
            ct_state[g] = [
                sm.tile([P, JT, GS, 32], F16, name="ct", tag="ct", bufs=4)
                for _ in range(2)
            ]
        ct_tiles = ct_state[g]
        if True:
            # s-einsum: 4 samples concurrent in one PSUM bank via col groups.
            ps_s = psum.tile([P, ND], F32, name="ps_s", tag="prt", bufs=3)
            if t == 0:
                # b=0 => c uniform: s_0 = (xsum @ W)/16 replicated over strips
                for kt in range(KT):
                    nc.tensor.matmul(
                        ps_s[:],
                        lhsT=XSt[:, g, kt, :],
                        rhs=wf[:, kt, :],
                        start=(kt == 0),
                        stop=(kt == KT - 1),
                    )
            else:
                ct = ct_tiles[t % 2]
                for jt in range(JT):
                    for a in range(GS):
                        nc.tensor.matmul(
                            ps_s[ts(a, 32), :],
                            lhsT=ct[:, jt, a, :],
                            rhs=uh_tiles[samples[a]][:, jt, :],
                            start=(jt == 0),
                            stop=(jt == JT - 1),
                            tile_position=(0, 32 * a),
                            skip_group_check=True,
                        )

            # mask to block diagonal; norms; rinv = (n2+eps)^-0.5 on ScalarE as
            # Exp(-0.5 * Ln(n2+eps)); junk rows produce garbage that stays put.
            masked = rt.tile([P, ND], F16, name="masked")
            for mh in range(2):
                nc.vector.tensor_tensor(
                    masked[:, ts(mh, 256)], ps_s[:, ts(mh, 256)],
                    mask[:, ts(mh, 256)], op=ALU.mult,
                )
            sq = rt.tile([P, ND], F16, name="sq")
            n2 = rt.tile([P, 1], F32, name="n2")
            nc.scalar.activation(sq[:], masked[:], AF.Square, accum_out=n2[:])
            # rinv = (n2+eps)^-0.5: magic-constant guess + 1 Newton step on DVE
            xe = rt.tile([P, 1], F32, name="xe")
            nc.vector.tensor_scalar(xe[:], n2[:], EPS, None, op0=ALU.add)
            xh = rt.tile([P, 1], F32, name="xh")
            nc.vector.tensor_scalar(xh[:], xe[:], 0.5, None, op0=ALU.mult)
            yt = rt.tile([P, 1], F32, name="yt")
            nc.vector.tensor_scalar(
                yt.bitcast(I32)[:], xe.bitcast(I32)[:], 1, None,
                op0=ALU.logical_shift_right,
            )
            nc.vector.tensor_scalar(
                yt.bitcast(I32)[:], yt.bitcast(I32)[:], 0x5F3759E0, None,
                op0=ALU.subtract,
            )
            nc.vector.tensor_scalar(
                yt.bitcast(I32)[:], yt.bitcast(I32)[:], -1, None,
                op0=ALU.bitwise_xor,
            )
            y2 = rt.tile([P, 1], F32, name="y2")
            nc.vector.tensor_tensor(y2[:], yt[:], yt[:], op=ALU.mult)
            nc.vector.tensor_tensor(y2[:], y2[:], xh[:], op=ALU.mult)
            nc.vector.tensor_scalar(y2[:], y2[:], -1.0, 1.5, op0=ALU.mult, op1=ALU.add)
            nc.vector.tensor_tensor(yt[:], yt[:], y2[:], op=ALU.mult)
            rinv = yt

            # block-diagonal V (unnormalized): PE transpose of masked
            pv = psum.tile([P, IKT * P], F16, name="pv", tag="pf16", bufs=2)
            for c in range(IKT):
                nc.tensor.transpose(pv[:, ts(c, P)], masked[:, ts(c, P)], ident[:])
            vblk = rt.tile([P, IKT, P], F16, name="vblk")
            for c in range(IKT):
                nc.scalar.copy(vblk[:, c, :], pv[:, ts(c, P)])

            if t == ROUTINGS - 1:
                # final squash output: diag-extract via matmul with Sel, scale,
                # write packed [128, 32]; host unpacks the 4 strips.
                ps_v = psum.tile([P, DK], F32, name="ps_v", tag="prt", bufs=3)
                for kt in range(IKT):
                    nc.tensor.matmul(
                        ps_v[:],
                        lhsT=vblk[:, kt, :],
                        rhs=sel[:],
                        start=(kt == 0),
                        stop=(kt == IKT - 1),
                    )
                vout = rt.tile([P, DK], F32, name="vout")
                nc.scalar.activation(vout[:], ps_v[:], AF.Copy, scale=rinv[:])
                nc.gpsimd.dma_start(out_ap[g], vout[:])
                return

            # b-update: b[i,j] = sum_k v u_hatT; exp(rinv*b) fused into the PSUM
            # evacuation. Junk rows exp to garbage; it never leaves them.
            bsc = rt.tile([P, 2, ND], F16, name="bsc")
            for jc in range(2):
                ps_b = psum.tile([P, ND], F32, name="ps_b", tag="prt", bufs=3)
                for kt in range(IKT):
                    for a in range(GS):
                        nc.tensor.matmul(
                            ps_b[ts(a, 32), :],
                            lhsT=vblk[:, kt, ts(a, 32)],
                            rhs=uhT_tiles[samples[a]][:, 4 * jc : 4 * jc + 4, kt, :],
                            start=(kt == 0),
                            stop=(kt == IKT - 1),
                            tile_position=(0, 32 * a),
                            skip_group_check=True,
                        )
                for eh in range(2):
                    nc.scalar.activation(
                        bsc[:, jc, ts(eh, 256)], ps_b[:, ts(eh, 256)],
                        AF.Exp, scale=rinv[:],
                    )

            # transpose to bT [j-part, (sample, i)] and softmax over i;
            # processed in jc-halves so half 0's normalize chain overlaps
            # half 1's exp + transposes (cuts the serial tail per iteration)
            pbt = psum.tile([P, JT, P], F16, name="pbt", tag="pf16", bufs=2)
            expT = pbt.rearrange("p t (s c) -> p t s c", c=32)[:, :, :, 0:NI]
            zsum = sm.tile([P, JT, GS], F32, name="zsum")
            rz = sm.tile([P, JT, GS], F32, name="rz")
            rzx = sm.tile([P, JT, GS, 32], F16, name="rzx")
            ct_next = ct_tiles[(t + 1) % 2]
            ctv = ct_next.rearrange("p t s c -> p t (s c)")
            for h in range(2):
                hs = slice(4 * h, 4 * h + 4)
                for jt in range(4 * h, 4 * h + 4):
                    nc.tensor.transpose(
                        pbt[:, jt, :], bsc[:, jt // 4, ts(jt % 4, P)], ident[:]
                    )
                nc.vector.tensor_reduce(
                    zsum[:, hs, :], expT[:, hs, :, :], axis=AX.X, op=ALU.add
                )
                nc.vector.reciprocal(rz[:, hs, :], zsum[:, hs, :])
                nc.vector.tensor_copy(
                    rzx[:, hs, :, :],
                    rz[:, hs, :].unsqueeze(3).broadcast_to([P, 4, GS, 32]),
                )
                nc.vector.tensor_tensor(
                    ctv[:, hs, :],
                    pbt[:, hs, :],
                    rzx.rearrange("p t s c -> p t (s c)")[:, hs, :],
                    op=ALU.mult,
                )

    # software-pipelined emission: engine queues are in-order, so emission
    # order is the schedule. stage1 of the next group is spread between
    # routing iterations; the last two groups' iterations are interleaved.
    for s in range(GS):
        stage1(s)
    sched = [
        ("r", 0, 0), ("s", 4), ("s", 5), ("r", 0, 1), ("s", 6), ("s", 7),
        ("r", 0, 2), ("r", 0, 3),
        ("r", 1, 0), ("s", 8), ("s", 9), ("r", 1, 1), ("s", 10), ("s", 11),
        ("r", 1, 2), ("r", 1, 3),
        ("r", 2, 0), ("s", 12), ("s", 13), ("r", 2, 1), ("s", 14), ("s", 15),
        ("r", 2, 2), ("r", 3, 0), ("r", 2, 3), ("r", 3, 1),
        ("r", 3, 2), ("r", 3, 3),
    ]
    for item in sched:
        if item[0] == "s":
            stage1(item[1])
        else:
            routing_iter(item[1], item[2])


def _np_consts():
    ident = np.eye(P, dtype=np.float16)
    sel = np.tile(np.eye(DK, dtype=np.float16), (IKT, 1))
    mask = np.zeros((P, ND), dtype=np.float16)
    for a in range(GS):
        for i in range(NI):
            mask[32 * a + i, DK * i : DK * (i + 1)] = 1.0
    return ident, sel, mask


@functools.cache
def _build_nc():
    from contextlib import ExitStack

    nc = bacc.Bacc(
        "TRN2",
        target_bir_lowering=False,
        debug=False,
        num_devices=NCORES,
    )
    xT_t = nc.dram_tensor("xT", [BSH, NK, NJ], F16, kind="ExternalInput")
    w_t = nc.dram_tensor("w", [NK, ND], F16, kind="ExternalInput")
    xs_t = nc.dram_tensor("xs", [P, NG, KT, P], F16, kind="ExternalInput")
    ident_t = nc.dram_tensor("ident", [P, P], F16, kind="ExternalInput")
    sel_t = nc.dram_tensor("sel", [P, DK], F16, kind="ExternalInput")
    mask_t = nc.dram_tensor("mask", [P, ND], F16, kind="ExternalInput")
    out_t = nc.dram_tensor("out", [NG, P, DK], F32, kind="ExternalOutput")

    with tile.TileContext(nc) as tc:
        with ExitStack() as ctx:
            _build_body(
                nc, tc,
                xT_t.ap(), w_t.ap(), xs_t.ap(), ident_t.ap(), sel_t.ap(),
                mask_t.ap(), out_t.ap(),
                ctx,
            )
    nc.compile()
    return nc


def _in_maps(x, W):
    x = np.asarray(x, dtype=np.float32)
    w2d = np.asarray(W, dtype=np.float32).reshape(NK, ND).astype(np.float16)
    ident, sel, mask = _np_consts()
    maps = []
    for c in range(NCORES):
        shard = x[c * BSH : (c + 1) * BSH]
        xT = np.ascontiguousarray(shard.transpose(0, 2, 1)).astype(np.float16)
        xsum = shard.sum(axis=1)  # [BSH, NK] fp32
        XS = np.zeros((P, NG, KT, P), np.float32)
        for g in range(NG):
            for a in range(GS):
                col = xsum[g * GS + a].reshape(KT, P).transpose(1, 0) / NI
                XS[:, g, :, 32 * a : 32 * a + NI] = col[:, :, None]
        maps.append(
            {
                "xT": xT,
                "w": np.ascontiguousarray(w2d),
                "xs": XS.astype(np.float16),
                "ident": ident,
                "sel": sel,
                "mask": mask,
            }
        )
    return maps


def run(x, W, trace=False):
    nc = _build_nc()
    res = run_bass_kernel_spmd(nc, _in_maps(x, W), list(range(NCORES)), trace=trace)
    outs = []
    for r in res.results:
        scr = r["out"]  # [NG, 128, 32]
        v = scr.reshape(NG, GS, 32, DK)[:, :, :NI, :].reshape(BSH, NI, DK)
        outs.append(v)
    out = np.concatenate(outs, axis=0)
    return out.astype(np.float32), res


def kernel(x, W):
    out, _ = run(x, W, trace=False)
    return out


# revision 34
# speedup vs baseline: 1.0559x; 1.0190x over previous
"""CapsuleLayer dynamic-routing kernel for Trainium2 (Bass/Tile), SPMD over 8 cores.

Math (per batch sample, from the reference):
    u_hat[j, (i,k)] = sum_k' x[j, k'] * W[k', (i,k)]        j=1024, k'=256, (i,k)=16x32=512
    b_0 = 0
    for t in 0..3:
        c = softmax_i(b)                                    [16, 1024]
        s[i, k] = sum_j c[i, j] * u_hat[j, (i,k)]
        v = s / sqrt(sum_k s^2 + eps)                       [16, 32]
        if t < 3: b[i, j] = sum_k v[i, k] * u_hat[j, (i,k)]
    return v

Sharding: data-parallel over batch (128 -> 16 per core), W replicated.

v3 layout strategy (all matmul inputs fp16, PSUM fp32):
  - x is pre-transposed AND pre-summed over j on the HOST: xT fp16 [256, 1024]
    uploads with plain DMAs (no device xbar transposes, no startup serialization).
  - u_hat [j-part, (ik)] via matmul lhsT=xT-chunks rhs=W; PSUM evacuated
    fp32->fp16 alternating Vector/Scalar.
  - u_hatT [(ik)-part, j] via ONE SBUF->SBUF DMA-crossbar transpose per sample
    on the sync HWDGE queue (scalar-queue DMAs block ScalarE compute); group 0
    uses PE transposes instead since the sync queue can't fill fast enough at
    startup.
  - routing iteration 0 exploits b=0 => c uniform: s_0 = (xsum @ W)/16, done as a
    single full-bank matmul against a host-built replicated-xsum stationary (XS);
    no c-tile init is needed anywhere.
  - routing processes 4 samples per group, packed 32-per-sample in PSUM
    partitions with col-group tile_position for concurrent PE strips; one
    accumulation group per bank (start=True only on the very first matmul).
  - every 32-row strip is its OWN PSUM accumulation group (start=True on its
    first matmul clears has_written for just that strip region), so no PSUM
    memsets / zero-inits are needed; junk rows (16..31 of each strip) stay
    finite and self-damping (exp of junk is exp(rinv*0)=1, the Z-sum only
    reduces real columns, and the mask zeroes junk before anything nonlinear),
    so no NaN/inf can ever contaminate real lanes.
  - rsqrt via DVE magic-constant + ONE Newton step (0.17% max err, well within
    tolerance); keeping Ln off ScalarE avoids activation-table thrash (only one
    table load for Copy/Square/Exp).
  - softmax runs in the bT layout [j-part, (sample, i)]; 1/Z applied as one flat
    [128,1024] multiply against a broadcast-expanded reciprocal.
  - final v written packed [128, 32] per group with one contiguous DMA to a
    DRAM scratch; host unpacks strips.
"""

import functools

import numpy as np

import concourse.bass as bass
import concourse.mybir as mybir
import concourse.tile as tile
from concourse import bacc
from concourse.bass_utils import run_bass_kernel_spmd

F32 = mybir.dt.float32
F16 = mybir.dt.float16
I32 = mybir.dt.int32
AF = mybir.ActivationFunctionType
ALU = mybir.AluOpType
AX = mybir.AxisListType
ts = bass.ts

NCORES = 8
BFULL = 128
BSH = BFULL // NCORES  # 16 samples per core
NJ, NK, ND = 1024, 256, 512  # j, k', (i,k)
NI, DK = 16, 32
JT, KT, IKT = NJ // 128, NK // 128, ND // 128  # 8, 2, 4
GS = 4  # samples per routing group (packed in PSUM partitions at 32-stride)
NG = BSH // GS  # 4
ROUTINGS = 4
EPS = 1e-7
P = 128


def _build_body(nc, tc, xT_ap, w_ap, xs_ap, ident_ap, sel_ap, mask_ap, out_ap, ctx):
    consts = ctx.enter_context(tc.tile_pool(name="consts", bufs=1))
    xTp = ctx.enter_context(tc.tile_pool(name="xT", bufs=6))
    uhp = ctx.enter_context(tc.tile_pool(name="uh", bufs=2 * GS))
    uhTp = ctx.enter_context(tc.tile_pool(name="uhT", bufs=2 * GS))
    rt = ctx.enter_context(tc.tile_pool(name="rt", bufs=3))
    sm = ctx.enter_context(tc.tile_pool(name="sm", bufs=3))
    psum = ctx.enter_context(tc.tile_pool(name="psum", bufs=2, space="PSUM"))

    # ---- input/const DMAs ----
    # gpsimd queue: all x uploads + most consts, in consumption order.
    # sync queue: W (gates the first matmul), then the even-sample uhT
    # transposes; scalar queue: odd-sample uhT transposes.  Keeping the
    # transposes off the x-upload queue and in consumption order avoids
    # head-of-line blocking on the HWDGE queues.
    xT_tiles = []
    for s in range(BSH):
        xTt = xTp.tile([P, KT, NJ], F16, name="xT")
        xT_tiles.append(xTt)

    wf = consts.tile([P, KT, ND], F16)
    ident = consts.tile([P, P], F16)
    sel = consts.tile([P, DK], F16)
    mask = consts.tile([P, ND], F16)
    XSt = consts.tile([P, NG, KT, P], F16)

    nc.sync.dma_start(wf[:], w_ap.rearrange("(t p) d -> p t d", p=P))
    for s in range(4):
        nc.gpsimd.dma_start(
            xT_tiles[s][:], xT_ap[s].rearrange("(t p) j -> p t j", p=P)
        )
    nc.gpsimd.dma_start(ident[:], ident_ap)
    nc.gpsimd.dma_start(mask[:], mask_ap)
    nc.gpsimd.dma_start(XSt[:], xs_ap)
    nc.gpsimd.dma_start(sel[:], sel_ap)
    for s in range(4, BSH):
        nc.gpsimd.dma_start(
            xT_tiles[s][:], xT_ap[s].rearrange("(t p) j -> p t j", p=P)
        )

    uh_tiles = [None] * BSH
    uhT_tiles = [None] * BSH

    def stage1(s):
        """xT[s] -> u_hat [j,(ik)] fp16 -> u_hatT via DMA xbar transpose."""
        xTt = xT_tiles[s]
        uh = uhp.tile([P, JT, ND], F16, name="uh")
        for jt in range(JT):
            pu = psum.tile([P, ND], F32, name="pu", tag="pu", bufs=3)
            for kt in range(KT):
                nc.tensor.matmul(
                    pu[:],
                    lhsT=xTt[:, kt, ts(jt, P)],
                    rhs=wf[:, kt, :],
                    start=(kt == 0),
                    stop=(kt == KT - 1),
                )
            if jt % 2 == 0:
                nc.vector.tensor_copy(uh[:, jt, :], pu[:])
            else:
                nc.scalar.copy(uh[:, jt, :], pu[:])

        # uhT[p, jt, dt, jl] = u_hat[128*jt + jl, 128*dt + p]
        uhT = uhTp.tile([P, JT, IKT, P], F16, name="uhT")
        if s < GS:
            # group 0: PE transposes (sync DMA queue can't fill fast enough
            # at startup); evac alternates Vector/Scalar
            for dt in range(IKT):
                pt2 = psum.tile([P, NJ], F16, name="pt2", tag="pf16", bufs=2)
                for jt in range(JT):
                    nc.tensor.transpose(
                        pt2[:, ts(jt, P)], uh[:, jt, ts(dt, P)], ident[:]
                    )
                if dt % 2 == 0:
                    nc.vector.tensor_copy(uhT[:, :, dt, :], pt2.rearrange("p (t c) -> p t c", c=P))
                else:
                    nc.scalar.copy(uhT[:, :, dt, :], pt2.rearrange("p (t c) -> p t c", c=P))
        else:
            # two jc-halves back to back: no head-of-line blocking (each half
            # only needs this sample's uh), and the jc=0 b-update can start
            # as soon as every sample's A-half is done.
            nc.sync.dma_start_transpose(
                uhT[:, 0:4, :, :].rearrange("p a b c -> p (a b) c"),
                uh[:, 0:4, :].rearrange("p t d -> p (t d)"),
            )
            nc.sync.dma_start_transpose(
                uhT[:, 4:8, :, :].rearrange("p a b c -> p (a b) c"),
                uh[:, 4:8, :].rearrange("p t d -> p (t d)"),
            )
        uh_tiles[s] = uh
        uhT_tiles[s] = uhT

    ct_state = {}

    def routing_iter(g, t):
        samples = [g * GS + i for i in range(GS)]
        if t == 0:
            # two cT tiles, alternating across iterations
            ct_state[g] = [
                sm.tile([P, JT, GS, 32], F16, name="ct", tag="ct", bufs=4)
                for _ in range(2)
            ]
        ct_tiles = ct_state[g]
        if True:
            # s-einsum: 4 samples concurrent in one PSUM bank via col groups.
            ps_s = psum.tile([P, ND], F32, name="ps_s", tag="prt", bufs=3)
            if t == 0:
                # b=0 => c uniform: s_0 = (xsum @ W)/16 replicated over strips
                for kt in range(KT):
                    nc.tensor.matmul(
                        ps_s[:],
                        lhsT=XSt[:, g, kt, :],
                        rhs=wf[:, kt, :],
                        start=(kt == 0),
                        stop=(kt == KT - 1),
                    )
            else:
                ct = ct_tiles[t % 2]
                for jt in range(JT):
                    for a in range(GS):
                        nc.tensor.matmul(
                            ps_s[ts(a, 32), :],
                            lhsT=ct[:, jt, a, :],
                            rhs=uh_tiles[samples[a]][:, jt, :],
                            start=(jt == 0),
                            stop=(jt == JT - 1),
                            tile_position=(0, 32 * a),
                            skip_group_check=True,
                        )

            # mask to block diagonal; norms; rinv = (n2+eps)^-0.5 on ScalarE as
            # Exp(-0.5 * Ln(n2+eps)); junk rows produce garbage that stays put.
            masked = rt.tile([P, ND], F16, name="masked")
            for mh in range(2):
                nc.vector.tensor_tensor(
                    masked[:, ts(mh, 256)], ps_s[:, ts(mh, 256)],
                    mask[:, ts(mh, 256)], op=ALU.mult,
                )
            sq = rt.tile([P, ND], F16, name="sq")
            n2 = rt.tile([P, 1], F32, name="n2")
            nc.scalar.activation(sq[:], masked[:], AF.Square, accum_out=n2[:])
            # rinv = (n2+eps)^-0.5: magic-constant guess + 1 Newton step on DVE
            xe = rt.tile([P, 1], F32, name="xe")
            nc.vector.tensor_scalar(xe[:], n2[:], EPS, None, op0=ALU.add)
            xh = rt.tile([P, 1], F32, name="xh")
            nc.vector.tensor_scalar(xh[:], xe[:], 0.5, None, op0=ALU.mult)
            yt = rt.tile([P, 1], F32, name="yt")
            nc.vector.tensor_scalar(
                yt.bitcast(I32)[:], xe.bitcast(I32)[:], 1, None,
                op0=ALU.logical_shift_right,
            )
            nc.vector.tensor_scalar(
                yt.bitcast(I32)[:], yt.bitcast(I32)[:], 0x5F3759E0, None,
                op0=ALU.subtract,
            )
            nc.vector.tensor_scalar(
                yt.bitcast(I32)[:], yt.bitcast(I32)[:], -1, None,
                op0=ALU.bitwise_xor,
            )
            y2 = rt.tile([P, 1], F32, name="y2")
            nc.vector.tensor_tensor(y2[:], yt[:], yt[:], op=ALU.mult)
            nc.vector.tensor_tensor(y2[:], y2[:], xh[:], op=ALU.mult)
            nc.vector.tensor_scalar(y2[:], y2[:], -1.0, 1.5, op0=ALU.mult, op1=ALU.add)
            nc.vector.tensor_tensor(yt[:], yt[:], y2[:], op=ALU.mult)
            rinv = yt

            # block-diagonal V (unnormalized): PE transpose of masked
            pv = psum.tile([P, IKT * P], F16, name="pv", tag="pf16", bufs=2)
            for c in range(IKT):
                nc.tensor.transpose(pv[:, ts(c, P)], masked[:, ts(c, P)], ident[:])
            vblk = rt.tile([P, IKT, P], F16, name="vblk")
            for c in range(IKT):
                nc.scalar.copy(vblk[:, c, :], pv[:, ts(c, P)])

            if t == ROUTINGS - 1:
                # final squash output: diag-extract via matmul with Sel, scale,
                # write packed [128, 32]; host unpacks the 4 strips.
                ps_v = psum.tile([P, DK], F32, name="ps_v", tag="prt", bufs=3)
                for kt in range(IKT):
                    nc.tensor.matmul(
                        ps_v[:],
                        lhsT=vblk[:, kt, :],
                        rhs=sel[:],
                        start=(kt == 0),
                        stop=(kt == IKT - 1),
                    )
                vout = rt.tile([P, DK], F32, name="vout")
                nc.scalar.activation(vout[:], ps_v[:], AF.Copy, scale=rinv[:])
                nc.gpsimd.dma_start(out_ap[g], vout[:])
                return

            # b-update: b[i,j] = sum_k v u_hatT; exp(rinv*b) fused into the PSUM
            # evacuation. Junk rows exp to garbage; it never leaves them.
            bsc = rt.tile([P, 2, ND], F16, name="bsc")
            for jc in range(2):
                ps_b = psum.tile([P, ND], F32, name="ps_b", tag="prt", bufs=3)
                for kt in range(IKT):
                    for a in range(GS):
                        nc.tensor.matmul(
                            ps_b[ts(a, 32), :],
                            lhsT=vblk[:, kt, ts(a, 32)],
                            rhs=uhT_tiles[samples[a]][:, 4 * jc : 4 * jc + 4, kt, :],
                            start=(kt == 0),
                            stop=(kt == IKT - 1),
                            tile_position=(0, 32 * a),
                            skip_group_check=True,
                        )
                for eh in range(2):
                    nc.scalar.activation(
                        bsc[:, jc, ts(eh, 256)], ps_b[:, ts(eh, 256)],
                        AF.Exp, scale=rinv[:],
                    )

            # transpose to bT [j-part, (sample, i)] and softmax over i;
            # processed in jc-halves so half 0's normalize chain overlaps
            # half 1's exp + transposes (cuts the serial tail per iteration)
            pbt = psum.tile([P, JT, P], F16, name="pbt", tag="pf16", bufs=2)
            expT = pbt.rearrange("p t (s c) -> p t s c", c=32)[:, :, :, 0:NI]
            zsum = sm.tile([P, JT, GS], F32, name="zsum")
            rz = sm.tile([P, JT, GS], F32, name="rz")
            rzx = sm.tile([P, JT, GS, 32], F16, name="rzx")
            ct_next = ct_tiles[(t + 1) % 2]
            ctv = ct_next.rearrange("p t s c -> p t (s c)")
            for h in range(2):
                hs = slice(4 * h, 4 * h + 4)
                for jt in range(4 * h, 4 * h + 4):
                    nc.tensor.transpose(
                        pbt[:, jt, :], bsc[:, jt // 4, ts(jt % 4, P)], ident[:]
                    )
                nc.vector.tensor_reduce(
                    zsum[:, hs, :], expT[:, hs, :, :], axis=AX.X, op=ALU.add
                )
                nc.vector.reciprocal(rz[:, hs, :], zsum[:, hs, :])
                nc.vector.tensor_copy(
                    rzx[:, hs, :, :],
                    rz[:, hs, :].unsqueeze(3).broadcast_to([P, 4, GS, 32]),
                )
                nc.vector.tensor_tensor(
                    ctv[:, hs, :],
                    pbt[:, hs, :],
                    rzx.rearrange("p t s c -> p t (s c)")[:, hs, :],
                    op=ALU.mult,
                )

    # software-pipelined emission: engine queues are in-order, so emission
    # order is the schedule. stage1 of the next group is spread between
    # routing iterations; the last two groups' iterations are interleaved.
    for s in range(GS):
        stage1(s)
    sched = [
        ("r", 0, 0), ("s", 4), ("s", 5), ("r", 0, 1), ("s", 6), ("s", 7),
        ("r", 0, 2), ("r", 0, 3),
        ("r", 1, 0), ("s", 8), ("s", 9), ("r", 1, 1), ("s", 10), ("s", 11),
        ("r", 1, 2), ("r", 1, 3),
        ("r", 2, 0), ("s", 12), ("s", 13), ("r", 2, 1), ("s", 14), ("s", 15),
        ("r", 2, 2), ("r", 3, 0), ("r", 2, 3), ("r", 3, 1),
        ("r", 3, 2), ("r", 3, 3),
    ]
    for item in sched:
        if item[0] == "s":
            stage1(item[1])
        else:
            routing_iter(item[1], item[2])


def _np_consts():
    ident = np.eye(P, dtype=np.float16)
    sel = np.tile(np.eye(DK, dtype=np.float16), (IKT, 1))
    mask = np.zeros((P, ND), dtype=np.float16)
    for a in range(GS):
        for i in range(NI):
            mask[32 * a + i, DK * i : DK * (i + 1)] = 1.0
    return ident, sel, mask


@functools.cache
def _build_nc():
    from contextlib import ExitStack

    nc = bacc.Bacc(
        "TRN2",
        target_bir_lowering=False,
        debug=False,
        num_devices=NCORES,
    )
    xT_t = nc.dram_tensor("xT", [BSH, NK, NJ], F16, kind="ExternalInput")
    w_t = nc.dram_tensor("w", [NK, ND], F16, kind="ExternalInput")
    xs_t = nc.dram_tensor("xs", [P, NG, KT, P], F16, kind="ExternalInput")
    ident_t = nc.dram_tensor("ident", [P, P], F16, kind="ExternalInput")
    sel_t = nc.dram_tensor("sel", [P, DK], F16, kind="ExternalInput")
    mask_t = nc.dram_tensor("mask", [P, ND], F16, kind="ExternalInput")
    out_t = nc.dram_tensor("out", [NG, P, DK], F32, kind="ExternalOutput")

    with tile.TileContext(nc) as tc:
        with ExitStack() as ctx:
            _build_body(
                nc, tc,
                xT_t.ap(), w_t.ap(), xs_t.ap(), ident_t.ap(), sel_t.ap(),
                mask_t.ap(), out_t.ap(),
                ctx,
            )
    nc.compile()
    return nc


def _in_maps(x, W):
    x = np.asarray(x, dtype=np.float32)
    w2d = np.asarray(W, dtype=np.float32).reshape(NK, ND).astype(np.float16)
    ident, sel, mask = _np_consts()
    maps = []
    for c in range(NCORES):
        shard = x[c * BSH : (c + 1) * BSH]
        xT = np.ascontiguousarray(shard.transpose(0, 2, 1)).astype(np.float16)
        xsum = shard.sum(axis=1)  # [BSH, NK] fp32
        XS = np.zeros((P, NG, KT, P), np.float32)
        for g in range(NG):
            for a in range(GS):
                col = xsum[g * GS + a].reshape(KT, P).transpose(1, 0) / NI
                XS[:, g, :, 32 * a : 32 * a + NI] = col[:, :, None]
        maps.append(
            {
                "xT": xT,
                "w": np.ascontiguousarray(w2d),
                "xs": XS.astype(np.float16),
                "ident": ident,
                "sel": sel,
                "mask": mask,
            }
        )
    return maps


def run(x, W, trace=False):
    nc = _build_nc()
    res = run_bass_kernel_spmd(nc, _in_maps(x, W), list(range(NCORES)), trace=trace)
    outs = []
    for r in res.results:
        scr = r["out"]  # [NG, 128, 32]
        v = scr.reshape(NG, GS, 32, DK)[:, :, :NI, :].reshape(BSH, NI, DK)
        outs.append(v)
    out = np.concatenate(outs, axis=0)
    return out.astype(np.float32), res


def kernel(x, W):
    out, _ = run(x, W, trace=False)
    return out
